# revision 1
# baseline (speedup 1.0000x reference)
"""MAP loss (per-pixel 3x3 Gaussian NLL) Trainium2 kernel.

loss = mean_{b,m,n}( 0.5*T' Sy^{-1} T + 0.5*log det Sy ),  T = (target-mu)[b,:,m,n]
with loss zeroed if max(0.5*T'Sy^{-1}T) > 1e7.

Sharding: pure data-parallel over the batch dim (16 batches -> 2 per core on
8 cores); each core returns [128, 24] partials which the host folds into the
scalar loss.

Per pixel: LDL' factorization of the symmetric 3x3, reformulated so every
pivot reciprocal is a fused custom-DVE op and no divisions/dets are needed:

    r0 = 1/s00            l10 = s01*r0   l20 = s02*r0   m1 = s01^2*r0
    r1 = 1/(s11 - m1)     f1  = s12 - l10*s02           l21 = f1*r1
    r2 = 1/(s22 - (s02^2*r0 + f1^2*r1))
    z1 = T1 - l10*T0      z2 = (T2 - l20*T0) - l21*z1
    t1 = 0.5*(T0^2*r0 + z1^2*r1 + z2^2*r2)
    logdet = -(ln r0 + ln r1 + ln r2)

Design notes (from baseline trace: Vector 107us / Scalar 101us / DMA 96us):

* All three inputs stream in as SWDGE cast-DMAs (fp32 HBM -> bf16 SBUF),
  halving SBUF footprint and enabling 2x-mode bf16 tensor_tensor ops.
* Sigma stays INTERLEAVED in SBUF. The single-use entries (s00/s11/s22)
  are consumed directly with stride-9 access patterns by 1x custom DVE
  ops (strided reads cost nothing extra at 1x); only the multi-use
  entries (s01/s02/s12) are deinterleaved into dense planes by the
  Scalar engine. This kills the baseline's 57us full deinterleave.
* Three registered custom DVE ops fuse the hot math:
    RECIP_SUB_NR1_MAP: out = approx 1/(in0-in1)  (NOT-seed + 1 Newton,
      ~0.17% max err, one 6-stage op)
    SQ_MUL_MAP:        out = in0^2 * in1
  plus the stock RECIPROCAL_APPROX_FAST (2-Newton) for r0, invoked
  directly so bf16 in/out is allowed (the DVE pipeline is fp32
  internally, so the exponent-flip seed still works).
* Squares on Scalar, q-fold via bf16 scalar_tensor_tensor (2x mode) with
  per-tile accum_out columns; logdet via 3 Scalar Ln ops on r0/r1/r2
  (sign folded on host). t1 >= 0 so the per-(tile,partition) q sums
  bound max(t1); host re-checks exactly only if the bound trips.
"""

import functools
import numpy as np

B, C, M, N = 16, 3, 512, 512
NCORES = 8
BS = B // NCORES          # batches per core
P = 128                   # SBUF partitions
Q = (M * N) // P          # pixels per partition per batch image (2048)
F = 1024                  # max pixels per partition per tile
# (batch, col offset, width): small tiles at the ends shrink the pipeline
# fill (first sigma DMA + deinterleave before Vector can start) and the
# drain (last tile's Vector chain after the last DMA byte lands).
TILES = [(0, 0, 512), (0, 512, 512), (0, 1024, 1024),
         (1, 0, 1024), (1, 1024, 512), (1, 1536, 512)]
NT = len(TILES)           # tiles per core
NPIX = B * M * N
T1_CLIP = 1e7

_REGISTERED = {}


def _register_dve_ops():
    """Register the fused custom-DVE ops (idempotent). Uses the documented
    extension point: append to dve_ops.OPS / _SUB_OPCODE_FOR_NAME, with
    uops_sha computed in-process (self-consistent by construction)."""
    if _REGISTERED:
        return _REGISTERED
    from operator import add
    from concourse.dve_spec import (
        Spec, Src0, Src1, C0, C1, C2, Bin, AluOp, sq, lower, _has_src1,
    )
    from concourse.dve_uop import DveOpSpec
    import concourse.dve_ops as dops

    def make(name, spec, subdim=False):
        if name in dops._SUB_OPCODE_FOR_NAME:
            return next(o for o in dops.OPS if o.name == name)
        row = len(dops.OPS) + dops._CUSTOM_DVE_ROW_BASE
        dops._SUB_OPCODE_FOR_NAME[name] = row
        shas = {}
        for ver in ("v3", "v4"):
            s = DveOpSpec(name=name, opcode=row, uops=lower(spec, ver=ver),
                          rd1_en=_has_src1(spec))
            shas[ver] = s.sha(ver)
        op = dops.DveOp(name, spec, subdim=subdim, uops_sha=shas)
        dops.OPS.append(op)
        dops.CUSTOM_DVE_SPECS[name] = spec
        return op

    # out = approx 1/(in0 - in1): bitwise-NOT exponent-flip seed + one
    # Newton pass, computed on the internal fp32 stream so bf16 inputs are
    # fine. Constants are the stock minimax pair (optimal for 1-NR too).
    _d = Src0 - Src1
    _nd = Bin(AluOp.BITWISE_NOT, _d, _d)
    _y0 = _nd * C0

    def _ref_recip_sub(in0, in1, c0, c1, c2):
        d = in0.astype(np.float32) - in1.astype(np.float32)
        nd = (~d.view(np.int32)).view(np.float32)
        y0 = nd * c0
        return y0 * (c1 - d * y0)

    _REGISTERED["recip_sub"] = make(
        "RECIP_SUB_NR1_MAP",
        Spec(body=_y0 * (C1 - _d * _y0), reference=_ref_recip_sub),
    )
    _REGISTERED["sq_mul"] = make(
        "SQ_MUL_MAP",
        Spec(body=sq(Src0) * Src1,
             reference=lambda in0, in1, c0, c1, c2:
             in0.astype(np.float32) ** 2 * in1.astype(np.float32)),
    )
    # accum_out = sum(in0^2 * in1 * imm2): the whole 0.5*x^2*r quadratic
    # term, squared+scaled+folded in one 1x pass (replaces a Scalar square
    # plus a scalar_tensor_tensor that measured 1x anyway)
    _REGISTERED["q_acc"] = make(
        "SQ_MUL_RED_MAP",
        Spec(body=sq(Src0) * Src1 * C2, accum=add,
             reference=lambda in0, in1, c0, c1, c2:
             in0.astype(np.float32) ** 2 * in1.astype(np.float32) * c2),
    )
    _REGISTERED["recip"] = dops.RECIPROCAL_APPROX_FAST
    _REGISTERED["consts"] = dops.RECIP_APPROX_FAST_CONSTS
    return _REGISTERED


def _emit_body(nc, tc, tgt, mu, sig, out):
    from concourse import mybir

    ops = _register_dve_ops()
    RC = ops["consts"]
    f32 = mybir.dt.float32
    f32d = f32
    bf16 = mybir.dt.bfloat16
    AF = mybir.ActivationFunctionType
    Alu = mybir.AluOpType
    v = nc.vector
    sc = nc.scalar

    with (
        tc.tile_pool(name="io", bufs=2) as iop,
        tc.tile_pool(name="wk", bufs=1) as wk,
        tc.tile_pool(name="acc", bufs=1) as accp,
    ):
        qsp = accp.tile([P, 3 * NT], f32, tag="qsp", bufs=1, name="qsp")
        ldp = accp.tile([P, 3 * NT], f32, tag="ldp", bufs=1, name="ldp")

        sig_f = sig.rearrange("b m n c d -> b (m n c d)")
        tgt_f = tgt.rearrange("b c m n -> b c (m n)")
        mu_f = mu.rearrange("b c m n -> b c (m n)")

        def bt(tag, bufs=1):
            return wk.tile([P, F], bf16, tag=tag, bufs=bufs, name=tag)

        def emit_dma(b, o, fi, split):
            """Sigma streams fp32 over HWDGE (keeps GpSimd's SWDGE queue
            short); target/mu arrive as per-channel SWDGE cast-DMAs into
            [P, F] bf16 tiles (large-tile slice reads measured ~2x slower
            on the DVE, so each channel gets its own tile). `split` halves
            the sigma DMA so the deinterleave starts at the half-way mark
            -- worth the extra DMA only for big tiles and for tile 0
            (pipeline fill)."""
            nh = 2 if split else 1
            fh = fi // nh
            sig_tile = iop.tile([P, 9 * F], f32d, tag="sig", bufs=2,
                                name="sig")
            sig_src = sig_f[b].rearrange("(p q) -> p q", p=P)
            for h in range(nh):
                nc.sync.dma_start(
                    out=sig_tile[:, h * 9 * fh:(h + 1) * 9 * fh],
                    in_=sig_src[:, (o + h * fh) * 9:(o + (h + 1) * fh) * 9],
                )
            return dict(sig=sig_tile, fi=fi, split=split)

        def emit_tm(b, o, fi):
            tgc, muc = [], []
            for c in range(3):
                tg = iop.tile([P, F], bf16, tag=f"tg{c}", bufs=3,
                              name=f"tg{c}")
                tsrc = tgt_f[b, c].rearrange("(p q) -> p q", p=P)
                nc.gpsimd.dma_start(out=tg[:, :fi], in_=tsrc[:, o:o + fi])
                tgc.append(tg)
            for c in range(3):
                mv = iop.tile([P, F], bf16, tag=f"mu{c}", bufs=3,
                              name=f"mu{c}")
                msrc = mu_f[b, c].rearrange("(p q) -> p q", p=P)
                nc.gpsimd.dma_start(out=mv[:, :fi], in_=msrc[:, o:o + fi])
                muc.append(mv)
            return dict(tgc=tgc, muc=muc)

        def emit_deint(st):
            """All six distinct sigma entries -> dense bf16 planes, split
            across the Scalar AND GpSimd engines (both otherwise have
            slack) and emitted per DMA half so the copies start as soon as
            each half lands. Dense planes keep the Vector chain in 2x mode
            and the custom ops at clean 1x."""
            nh = 2 if st["split"] else 1
            fh = st["fi"] // nh
            planes = {k: bt(k, bufs=2)
                      for k in ("s00", "s01", "s02", "s11", "s12", "s22")}
            for h in range(nh):
                sv = st["sig"][:, h * 9 * fh:(h + 1) * 9 * fh].rearrange(
                    "p (f k) -> p f k", k=9
                )
                half = slice(h * fh, (h + 1) * fh)
                for key, j in (("s00", 0), ("s01", 1), ("s02", 2),
                               ("s11", 4), ("s12", 5), ("s22", 8)):
                    sc.copy(planes[key][:, half], sv[:, :, j])
            st.update(planes)

        def emit_compute(ti, st):
            fi = st["fi"]

            def w(tile_):
                return tile_[:, :fi]

            s00, s01, s02 = w(st["s00"]), w(st["s01"]), w(st["s02"])
            s11, s12, s22 = w(st["s11"]), w(st["s12"]), w(st["s22"])
            tgc = [w(t_) for t_ in st["tgc"]]
            muc = [w(t_) for t_ in st["muc"]]

            def nt(tag, bufs=1):
                return w(bt(tag, bufs=bufs))

            # ---- residual first: frees the tg/mu DMA ring slots early so
            # the (FIFO) DMA queues never head-of-line block on them
            T0, T1, T2 = nt("T0", bufs=2), nt("T1"), nt("T2")
            v.tensor_sub(T0, tgc[0], muc[0])
            v.tensor_sub(T1, tgc[1], muc[1])
            v.tensor_sub(T2, tgc[2], muc[2])

            # ---- LDL' factorization (fused custom DVE ops on dense planes)
            r0 = nt("r0", bufs=2)
            v._custom_dve(ops["recip"], out=r0, in0=s00,
                          s0=RC["s0"], s1=RC["s1"], imm2=RC["imm2"])
            l10 = nt("l10")
            v.tensor_mul(l10, s01, r0)
            l20 = nt("l20")
            v.tensor_mul(l20, s02, r0)
            m1 = nt("m1")
            v._custom_dve(ops["sq_mul"], out=m1, in0=s01, in1=r0)
            r1 = nt("r1", bufs=2)
            v._custom_dve(ops["recip_sub"], out=r1, in0=s11,
                          in1=m1, s0=RC["s0"], s1=RC["s1"])
            m2 = nt("m2")
            v.tensor_mul(m2, l10, s02)
            f1 = nt("f1")
            v.tensor_sub(f1, s12, m2)
            l21 = nt("l21")
            v.tensor_mul(l21, f1, r1)
            m3 = nt("m3")
            v._custom_dve(ops["sq_mul"], out=m3, in0=s02, in1=r0)
            m4 = nt("m4")
            v._custom_dve(ops["sq_mul"], out=m4, in0=f1, in1=r1)
            e2 = nt("e2")
            v.tensor_add(e2, m3, m4)
            r2 = nt("r2", bufs=2)
            v._custom_dve(ops["recip_sub"], out=r2, in0=s22,
                          in1=e2, s0=RC["s0"], s1=RC["s1"])

            # ---- Scalar: logdet contribution ln(r_i), summed along the
            # free dim into per-tile columns (sign folded on host)
            lnscr = nt("lnscr", bufs=2)
            sc.activation(lnscr, r0, AF.Ln,
                          accum_out=ldp[:, 3 * ti:3 * ti + 1])
            sc.activation(lnscr, r1, AF.Ln,
                          accum_out=ldp[:, 3 * ti + 1:3 * ti + 2])
            sc.activation(lnscr, r2, AF.Ln,
                          accum_out=ldp[:, 3 * ti + 2:3 * ti + 3])

            # ---- forward substitution (bf16 2x tensor_tensor)
            m5 = nt("m5")
            v.tensor_mul(m5, l10, T0)
            z1 = nt("z1", bufs=2)
            v.tensor_sub(z1, T1, m5)
            m6 = nt("m6")
            v.tensor_mul(m6, l20, T0)
            h0 = nt("h0")
            v.tensor_sub(h0, T2, m6)
            m7 = nt("m7")
            v.tensor_mul(m7, l21, z1)
            z2 = nt("z2", bufs=2)
            v.tensor_sub(z2, h0, m7)

            # ---- q-fold: fused 0.5*x^2*r with free-dim sum into per-tile
            # columns (one custom op per term)
            qscr = nt("qscr", bufs=2)
            for i, (x, r) in enumerate(((T0, r0), (z1, r1), (z2, r2))):
                v._custom_dve(
                    ops["q_acc"], out=qscr, in0=x, in1=r, imm2=0.5,
                    accum_out=qsp[:, 3 * ti + i:3 * ti + i + 1],
                )

        # Two-tile-ahead software pipeline. Interleaving the DMA issues
        # with the deint copies keeps both FIFO DMA queues fed ahead of
        # the compute that releases their ring slots.
        def sig_stage(ti):
            b, o, fi = TILES[ti]
            return emit_dma(b, o, fi, split=(fi >= 1024 or ti == 0))

        def tm_stage(ti):
            b, o, fi = TILES[ti]
            return emit_tm(b, o, fi)

        # Warm the ACT function-table set (copy/ln) during the pipeline
        # fill instead of at the first deinterleave copy.
        pre = accp.tile([P, 1], f32, tag="pre", bufs=1, name="pre")
        pre2 = accp.tile([P, 1], f32, tag="pre2", bufs=1, name="pre2")
        v.memset(pre[:], 1.0)
        sc.activation(pre2[:], pre[:], AF.Ln)

        # Sigma streams two tiles ahead, target/mu only one: during the
        # fill the sigma DMAs (which gate deint -> the whole Vector chain)
        # get the HBM bandwidth to themselves.
        sig_sts = [sig_stage(0), sig_stage(1)]
        tm_sts = [tm_stage(0), tm_stage(1)]
        emit_deint(sig_sts[0])
        for ti in range(NT):
            if ti + 2 < NT:
                sig_sts.append(sig_stage(ti + 2))
                tm_sts.append(tm_stage(ti + 2))
            if ti + 1 < NT:
                emit_deint(sig_sts[ti + 1])
            emit_compute(ti, {**sig_sts[ti], **tm_sts[ti]})

        nc.sync.dma_start(out=out[:, 0:3 * NT], in_=qsp[:])
        nc.sync.dma_start(out=out[:, 3 * NT:6 * NT], in_=ldp[:])


@functools.lru_cache(maxsize=1)
def _build():
    import concourse.bacc as bacc
    import concourse.tile as tile
    from concourse import mybir

    _register_dve_ops()
    f32 = mybir.dt.float32
    nc = bacc.Bacc("TRN2", target_bir_lowering=False, debug=False)
    tgt = nc.dram_tensor("target_s", [BS, C, M, N], f32, kind="ExternalInput").ap()
    mu = nc.dram_tensor("mu_s", [BS, C, M, N], f32, kind="ExternalInput").ap()
    sig = nc.dram_tensor("sigma_s", [BS, M, N, C, C], f32, kind="ExternalInput").ap()
    out = nc.dram_tensor("partials", [P, 6 * NT], f32, kind="ExternalOutput").ap()
    with tile.TileContext(nc) as tc:
        _emit_body(nc, tc, tgt, mu, sig, out)
    nc.compile()
    return nc


def _run_on_device(target, mu, sigma_y, trace=False):
    from concourse.bass_utils import run_bass_kernel_spmd

    nc = _build()
    target = np.ascontiguousarray(target, dtype=np.float32)
    mu = np.ascontiguousarray(mu, dtype=np.float32)
    sigma_y = np.ascontiguousarray(sigma_y, dtype=np.float32)
    in_maps = [
        {
            "target_s": target[i * BS:(i + 1) * BS],
            "mu_s": mu[i * BS:(i + 1) * BS],
            "sigma_s": sigma_y[i * BS:(i + 1) * BS],
        }
        for i in range(NCORES)
    ]
    return run_bass_kernel_spmd(nc, in_maps, list(range(NCORES)), trace=trace)


def kernel(target, mu, sigma_mu, sigma_n, sigma_y):
    res = _run_on_device(target, mu, sigma_y)
    partials = [res.results[i]["partials"] for i in range(NCORES)]
    sum_q = sum(p[:, 0:3 * NT].astype(np.float64).sum() for p in partials)
    sum_lr = sum(p[:, 3 * NT:6 * NT].astype(np.float64).sum() for p in partials)
    # per-(tile,partition) q sums bound max(t1) since every q term >= 0
    bound = max(
        p[:, 3 * ti:3 * ti + 3].astype(np.float64).sum(axis=1).max()
        for p in partials for ti in range(NT)
    )
    loss = np.float32((sum_q - 0.5 * sum_lr) / NPIX)
    if bound > T1_CLIP:
        # Upper bound tripped: pay for the exact host-side check.
        t = np.transpose(
            (target - mu).astype(np.float64), (0, 2, 3, 1)
        )[..., :, None]
        sol = np.linalg.solve(sigma_y.astype(np.float64), t)
        t1 = 0.5 * np.einsum("bmnci,bmnci->bmn", t, sol)
        if t1.max() > T1_CLIP:
            loss = np.float32(0.0)
    return loss

